# revision 1
# baseline (speedup 1.0000x reference)
"""GNN ensemble MoE-routing kernel for Trainium2 (8 NeuronCores).

Reference computes all 8 expert MLPs for every sample then selects one
(8x wasted FLOPs). This kernel routes on the host instead: samples are
gathered per expert, and core c runs ONLY expert c's MLP over the
samples routed to it (expert-parallel sharding).

Math folding (exact):
  lat = eps*sigma_c + mu_c  =>  lat @ W1_c = eps @ (sigma_c*W1_c) + mu_c@W1_c
so the device computes  sigmoid(eps @ W1p + b1p) @ W2 + b2  with
  W1p = sigma_c * W1_c,  b1p = b1_c + mu_c @ W1_c  (folded on host).

Device layout: features on SBUF partitions, samples on the free axis.
  epsT [512, K]  ->  H^T [1024, K]  ->  Y^T [512, K]
Weights load in natural layout as matmul lhsT (out = lhsT.T @ rhs), the
b1 bias+sigmoid fuse into one ScalarE activation (per-partition bias),
b2 is a DVE tensor_scalar add. Matmuls run as float32r (1 cycle/row for
moving dim >= 256, vs 4 cycles/row for plain fp32).
"""

from contextlib import ExitStack

import numpy as np

import concourse.bass as bass
import concourse.tile as tile
from concourse import bacc, mybir
from concourse.bass_utils import run_bass_kernel_spmd

NB_COMP = 8
LAT_DIM = 512
NB_NEUR = 1024
OUT_DIM = 512
N_CORES = 8

F32 = mybir.dt.float32
F32R = mybir.dt.float32r
SIG = mybir.ActivationFunctionType.Sigmoid
N_WARMUP = 13

_program_cache = {}


def _make_chunks(k_cap):
    # First chunk is 256 so the first matmul's DMA dependency (w1[0] +
    # x0[0]) is small and the PE starts early; the rest are 512-wide.
    chunks = []
    n0 = 0
    while n0 < k_cap:
        if n0 == 0 and k_cap % 512 == 256:
            ns = 256
        else:
            ns = min(512, k_cap - n0)
            if ns % 256 != 0:
                ns = 256 if k_cap - n0 <= 256 else ns
        chunks.append((n0, ns))
        n0 += ns
    return chunks


def _build_program(k_cap):
    """One-expert MLP over k_cap samples; same program runs SPMD on all 8 cores."""
    chunks = _make_chunks(k_cap)
    KC1, MC1 = LAT_DIM // 128, NB_NEUR // 128  # 4, 8
    KC2, MC2 = NB_NEUR // 128, OUT_DIM // 128  # 8, 4

    nc = bacc.Bacc(
        "TRN2",
        target_bir_lowering=False,
        debug=False,
        enable_asserts=False,
        num_devices=N_CORES,
    )
    epsT = nc.dram_tensor("epsT", [LAT_DIM, k_cap], F32R, kind="ExternalInput").ap()
    w1 = nc.dram_tensor("w1", [LAT_DIM, NB_NEUR], F32R, kind="ExternalInput").ap()
    b1 = nc.dram_tensor("b1", [128, MC1], F32, kind="ExternalInput").ap()
    w2 = nc.dram_tensor("w2", [NB_NEUR, OUT_DIM], F32R, kind="ExternalInput").ap()
    b2 = nc.dram_tensor("b2", [128, MC2], F32, kind="ExternalInput").ap()
    yT = nc.dram_tensor("yT", [OUT_DIM, k_cap], F32, kind="ExternalOutput").ap()

    with tile.TileContext(nc) as tc, ExitStack() as ctx:
        wpool = ctx.enter_context(tc.tile_pool(name="weights", bufs=1))
        xpool = ctx.enter_context(tc.tile_pool(name="x", bufs=3))
        hpool = ctx.enter_context(tc.tile_pool(name="h", bufs=2))
        ypool = ctx.enter_context(tc.tile_pool(name="y", bufs=6))
        # One shared pool holding all 8 PSUM banks; mm1 keeps 8 accumulators
        # live (kc-outer order), mm2 4, cycling through the same slots.
        pspool = ctx.enter_context(tc.tile_pool(name="ps", bufs=8, space="PSUM"))

        # DMA issue order matters twice over: HWDGE queues are FIFO, and
        # every dma_start costs a ~0.6us serialized DIRECT2D descriptor
        # write on the Sync sequencer (the only engine with the fast HWDGE
        # path - gpsimd/scalar DMAs measured slower). First chunk's inputs
        # lead so the PE starts as early as possible.
        first_xt = []
        w1t = []
        n0_0, ns_0 = chunks[0]
        for kc in range(KC1):
            t = wpool.tile([128, NB_NEUR], F32R, tag=f"w1_{kc}")
            nc.sync.dma_start(t[:], w1[kc * 128 : (kc + 1) * 128, :])
            w1t.append(t)
            tx = xpool.tile([128, ns_0], F32R, tag=f"x{kc}")
            nc.sync.dma_start(
                tx[:], epsT[kc * 128 : (kc + 1) * 128, n0_0 : n0_0 + ns_0]
            )
            first_xt.append(tx)
        b1t = wpool.tile([128, MC1], F32, tag="b1")
        nc.sync.dma_start(b1t[:], b1[:])
        w2t = []
        for kc in range(KC2):
            t = wpool.tile([128, OUT_DIM], F32R, tag=f"w2_{kc}")
            nc.sync.dma_start(t[:], w2[kc * 128 : (kc + 1) * 128, :])
            w2t.append(t)
        b2t = wpool.tile([128, MC2], F32, tag="b2")
        nc.sync.dma_start(b2t[:], b2[:])

        for ci, (n0, ns) in enumerate(chunks):
            if ci == 0:
                xt = first_xt
            else:
                xt = []
                for kc in range(KC1):
                    t = xpool.tile([128, ns], F32R, tag=f"x{kc}")
                    nc.sync.dma_start(
                        t[:], epsT[kc * 128 : (kc + 1) * 128, n0 : n0 + ns]
                    )
                    xt.append(t)

            ht = []
            ps1 = [
                pspool.tile([128, ns], F32, tag="ps", name=f"ps1_{ci}_{i}")
                for i in range(MC1)
            ]
            for kc in range(KC1):
                for mc in range(MC1):
                    nc.tensor.matmul(
                        ps1[mc][:],
                        w1t[kc][:, mc * 128 : (mc + 1) * 128],
                        xt[kc][:],
                        start=(kc == 0),
                        stop=(kc == KC1 - 1),
                    )
                    if kc == KC1 - 1:
                        h = hpool.tile([128, ns], F32R, tag=f"h{mc}")
                        nc.scalar.activation(
                            h[:], ps1[mc][:], SIG, bias=b1t[:, mc : mc + 1]
                        )
                        ht.append(h)

            ps2 = [
                pspool.tile([128, ns], F32, tag="ps", name=f"ps2_{ci}_{i}")
                for i in range(MC2)
            ]
            for kc in range(KC2):
                for oc in range(MC2):
                    nc.tensor.matmul(
                        ps2[oc][:],
                        w2t[kc][:, oc * 128 : (oc + 1) * 128],
                        ht[kc][:],
                        start=(kc == 0),
                        stop=(kc == KC2 - 1),
                    )
                    if kc == KC2 - 1:
                        y = ypool.tile([128, ns], F32, tag="y")
                        nc.vector.tensor_scalar_add(
                            y[:], ps2[oc][:], b2t[:, oc : oc + 1]
                        )
                        nc.sync.dma_start(
                            yT[oc * 128 : (oc + 1) * 128, n0 : n0 + ns], y[:]
                        )

    nc.compile()
    return nc


def get_program(k_cap):
    if k_cap not in _program_cache:
        _program_cache[k_cap] = _build_program(k_cap)
    return _program_cache[k_cap]


def _softplus(x):
    x = x.astype(np.float64)
    return (np.maximum(x, 0.0) + np.log1p(np.exp(-np.abs(x)))).astype(np.float32)


def kernel(epsilon, comp_idx, mu, rho, W1, b1, W2, b2, _trace=False):
    epsilon = np.asarray(epsilon, dtype=np.float32)
    comp_idx = np.asarray(comp_idx, dtype=np.int32)
    mu = np.asarray(mu, dtype=np.float32)
    rho = np.asarray(rho, dtype=np.float32)
    W1 = np.asarray(W1, dtype=np.float32)
    b1 = np.asarray(b1, dtype=np.float32)
    W2 = np.asarray(W2, dtype=np.float32)
    b2 = np.asarray(b2, dtype=np.float32)

    n = epsilon.shape[0]
    sigma = _softplus(rho)  # [C]

    sels = [np.nonzero(comp_idx == c)[0] for c in range(NB_COMP)]
    counts = [len(s) for s in sels]
    k_cap = max(256, -(-max(counts) // 256) * 256)

    nc = get_program(k_cap)

    in_maps = []
    for c in range(NB_COMP):
        sel = sels[c]
        epsT = np.zeros((LAT_DIM, k_cap), dtype=np.float32)
        if len(sel):
            epsT[:, : len(sel)] = epsilon[sel].T
        w1p = (W1[c] * sigma[c]).astype(np.float32)
        b1p = (b1[c].astype(np.float64) + mu[c].astype(np.float64) @ W1[c].astype(np.float64)).astype(np.float32)
        in_maps.append(
            {
                "epsT": epsT,
                "w1": np.ascontiguousarray(w1p),
                "b1": np.ascontiguousarray(b1p.reshape(NB_NEUR // 128, 128).T),
                "w2": np.ascontiguousarray(W2[c]),
                "b2": np.ascontiguousarray(b2[c].reshape(OUT_DIM // 128, 128).T),
            }
        )

    res = run_bass_kernel_spmd(
        nc,
        in_maps,
        core_ids=list(range(N_CORES)),
        trace=_trace,
        trace_cores=list(range(N_CORES)) if _trace else None,
    )

    out = np.zeros((n, OUT_DIM), dtype=np.float32)
    for c in range(NB_COMP):
        sel = sels[c]
        if len(sel):
            out[sel] = res.results[c]["yT"][:, : len(sel)].T
    if _trace:
        return out, res
    return out



# revision 2
# speedup vs baseline: 1.4460x; 1.4460x over previous
"""GNN ensemble MoE-routing kernel for Trainium2 (8 NeuronCores).

Reference computes all 8 expert MLPs for every sample then selects one
(8x wasted FLOPs). This kernel routes on the host instead: samples are
gathered per expert, and core c runs ONLY expert c's MLP over the
samples routed to it (expert-parallel sharding).

Math folding (exact):
  lat = eps*sigma_c + mu_c  =>  lat @ W1_c = eps @ (sigma_c*W1_c) + mu_c@W1_c
so the device computes  sigmoid(eps @ W1p + b1p) @ W2 + b2  with
  W1p = sigma_c * W1_c,  b1p = b1_c + mu_c @ W1_c  (folded on host).

Precision/perf layout (rel-err budget 2e-2; measured ~1.1e-2):
  mm1 runs in fp8 e4m3 with perf_mode=DoubleRow (2 contraction rows
  packed per PE cell, ~1.44x bf16 throughput at 512-wide moving dim).
  eps quantizes to e4m3 raw (std 1.0); W1p is scaled x64 before
  quantization so its ~0.03-std values stay in e4m3's normal range,
  and the 1/64 rescale folds into the sigmoid activation's scale
  operand (sigmoid(psum/64 + b1p)). mm2 runs in bf16 (same PE speed
  as f32r, half the DMA bytes; ~0.3% extra error).

Device layout: features on SBUF partitions, samples on the free axis.
DoubleRow operands are [128, K_blocks, free] with contraction index
k = block*128 + partition, so host tensors pack as
reshape(blocks,128,cols).transpose(1,0,2) -> one contiguous DMA each.
k_cap is the max per-expert count rounded to 16 (not 256): chunks are
near-equal multiples of 16 in (256, 512], all full-speed for f32r/bf16.
"""

from contextlib import ExitStack

import ml_dtypes
import numpy as np

import concourse.bass as bass
import concourse.tile as tile
from concourse import bacc, mybir
from concourse.bass_utils import run_bass_kernel_spmd

NB_COMP = 8
LAT_DIM = 512
NB_NEUR = 1024
OUT_DIM = 512
N_CORES = 8

F32 = mybir.dt.float32
F32R = mybir.dt.float32r
BF16 = mybir.dt.bfloat16
FP8 = mybir.dt.float8e4
SIG = mybir.ActivationFunctionType.Sigmoid
DR = mybir.MatmulPerfMode.DoubleRow

E4M3 = ml_dtypes.float8_e4m3
NPBF16 = ml_dtypes.bfloat16
W1_SCALE = 64.0

_program_cache = {}


def _make_chunks(k_cap):
    """Near-equal chunks, multiples of 16, each <=512 (PSUM bank / moving
    dim limit) and >=256 when possible (f32r full-rate floor for mm2)."""
    n_chunks = -(-k_cap // 512)
    base = (k_cap // n_chunks) // 16 * 16
    sizes = [base] * n_chunks
    sizes[-1] += k_cap - base * n_chunks
    chunks = []
    n0 = 0
    for ns in sizes:
        chunks.append((n0, ns))
        n0 += ns
    return chunks


def _build_program(k_cap):
    """One-expert MLP over k_cap samples; same program runs SPMD on all 8 cores."""
    chunks = _make_chunks(k_cap)
    KB1 = LAT_DIM // 128   # 4 contraction blocks for mm1
    MC1 = NB_NEUR // 128   # 8 output tiles for mm1
    KC2 = NB_NEUR // 128   # 8 contraction blocks for mm2
    MC2 = OUT_DIM // 128   # 4 output tiles for mm2

    nc = bacc.Bacc(
        "TRN2",
        target_bir_lowering=False,
        debug=False,
        enable_asserts=False,
        num_devices=N_CORES,
    )
    epsT = nc.dram_tensor("epsT", [128, KB1, k_cap], FP8, kind="ExternalInput").ap()
    w1 = nc.dram_tensor("w1", [128, KB1, NB_NEUR], FP8, kind="ExternalInput").ap()
    b1 = nc.dram_tensor("b1", [128, MC1], F32, kind="ExternalInput").ap()
    w2 = nc.dram_tensor("w2", [128, KC2, OUT_DIM], BF16, kind="ExternalInput").ap()
    b2 = nc.dram_tensor("b2", [128, MC2], F32, kind="ExternalInput").ap()
    yT = nc.dram_tensor("yT", [OUT_DIM, k_cap], F32, kind="ExternalOutput").ap()

    with tile.TileContext(nc) as tc, ExitStack() as ctx:
        wpool = ctx.enter_context(tc.tile_pool(name="weights", bufs=1))
        xpool = ctx.enter_context(tc.tile_pool(name="x", bufs=3))
        hpool = ctx.enter_context(tc.tile_pool(name="h", bufs=2))
        ypool = ctx.enter_context(tc.tile_pool(name="y", bufs=6))
        # One shared pool holding all 8 PSUM banks; mm1 keeps 8 accumulators
        # live, mm2 4, cycling through the same slots.
        pspool = ctx.enter_context(tc.tile_pool(name="ps", bufs=8, space="PSUM"))

        # DMA issue order matters twice over: HWDGE queues are FIFO, and
        # every dma_start costs a ~0.6us serialized DIRECT2D descriptor
        # write on the Sync sequencer. First-matmul deps (w1, x0) lead so
        # the PE starts as early as possible; each logical tensor is one
        # batched DMA (128 descriptors of contiguous rows).
        w1t = wpool.tile([128, KB1, NB_NEUR], FP8, tag="w1")
        nc.sync.dma_start(w1t[:], w1[:])
        n0_0, ns_0 = chunks[0]
        x0 = xpool.tile([128, KB1, ns_0], FP8, tag="x")
        nc.sync.dma_start(x0[:], epsT[:, :, n0_0 : n0_0 + ns_0])
        b1t = wpool.tile([128, MC1], F32, tag="b1")
        nc.sync.dma_start(b1t[:], b1[:])
        w2t = wpool.tile([128, KC2, OUT_DIM], BF16, tag="w2")
        nc.sync.dma_start(w2t[:], w2[:])
        b2t = wpool.tile([128, MC2], F32, tag="b2")
        nc.sync.dma_start(b2t[:], b2[:])

        for ci, (n0, ns) in enumerate(chunks):
            if ci == 0:
                xt = x0
            else:
                xt = xpool.tile([128, KB1, ns], FP8, tag="x")
                nc.sync.dma_start(xt[:], epsT[:, :, n0 : n0 + ns])

            # mm1: fp8 DoubleRow, contraction 512 = 2 groups x (2 blocks
            # packed per cell x 128 partitions). mc-outer so each ps1[mc]
            # finishes consecutively and its sigmoid overlaps the rest of
            # mm1 on the Scalar engine.
            ht = []
            ps1 = [
                pspool.tile([128, ns], F32, tag="ps", name=f"ps1_{ci}_{i}")
                for i in range(MC1)
            ]
            for mc in range(MC1):
                for g in range(2):
                    nc.tensor.matmul(
                        ps1[mc][:],
                        w1t[:, 2 * g : 2 * g + 2, mc * 128 : (mc + 1) * 128],
                        xt[:, 2 * g : 2 * g + 2, :],
                        start=(g == 0),
                        stop=(g == 1),
                        perf_mode=DR,
                    )
                h = hpool.tile([128, ns], BF16, tag=f"h{mc}")
                nc.scalar.activation(
                    h[:], ps1[mc][:], SIG,
                    bias=b1t[:, mc : mc + 1], scale=1.0 / W1_SCALE,
                )
                ht.append(h)

            # mm2: bf16, kc-outer; h[kc] for late kc is ready well before
            # its contraction step, so the PE never stalls on the Scalar
            # engine.
            ps2 = [
                pspool.tile([128, ns], F32, tag="ps", name=f"ps2_{ci}_{i}")
                for i in range(MC2)
            ]
            for kc in range(KC2):
                for oc in range(MC2):
                    nc.tensor.matmul(
                        ps2[oc][:],
                        w2t[:, kc, oc * 128 : (oc + 1) * 128],
                        ht[kc][:],
                        start=(kc == 0),
                        stop=(kc == KC2 - 1),
                    )
                    if kc == KC2 - 1:
                        y = ypool.tile([128, ns], F32, tag="y")
                        nc.vector.tensor_scalar_add(
                            y[:], ps2[oc][:], b2t[:, oc : oc + 1]
                        )
                        nc.sync.dma_start(
                            yT[oc * 128 : (oc + 1) * 128, n0 : n0 + ns], y[:]
                        )

    nc.compile()
    return nc


def get_program(k_cap):
    if k_cap not in _program_cache:
        _program_cache[k_cap] = _build_program(k_cap)
    return _program_cache[k_cap]


def _softplus(x):
    x = x.astype(np.float64)
    return (np.maximum(x, 0.0) + np.log1p(np.exp(-np.abs(x)))).astype(np.float32)


def _pack_blocks(a, nblk):
    """[nblk*128, C] -> [128, nblk, C] with out[p, b, c] = a[b*128+p, c]."""
    return np.ascontiguousarray(
        a.reshape(nblk, 128, a.shape[1]).transpose(1, 0, 2)
    )


def kernel(epsilon, comp_idx, mu, rho, W1, b1, W2, b2, _trace=False):
    epsilon = np.asarray(epsilon, dtype=np.float32)
    comp_idx = np.asarray(comp_idx, dtype=np.int32)
    mu = np.asarray(mu, dtype=np.float32)
    rho = np.asarray(rho, dtype=np.float32)
    W1 = np.asarray(W1, dtype=np.float32)
    b1 = np.asarray(b1, dtype=np.float32)
    W2 = np.asarray(W2, dtype=np.float32)
    b2 = np.asarray(b2, dtype=np.float32)

    n = epsilon.shape[0]
    sigma = _softplus(rho)  # [C]

    sels = [np.nonzero(comp_idx == c)[0] for c in range(NB_COMP)]
    counts = [len(s) for s in sels]
    k_cap = max(256, -(-max(counts) // 16) * 16)

    nc = get_program(k_cap)

    eps_q = epsilon.astype(E4M3)  # quantize once; std ~1 sits mid e4m3 range
    in_maps = []
    for c in range(NB_COMP):
        sel = sels[c]
        epsT = np.zeros((128, LAT_DIM // 128, k_cap), dtype=E4M3)
        if len(sel):
            epsT[:, :, : len(sel)] = _pack_blocks(
                eps_q[sel].T, LAT_DIM // 128
            )
        w1p = (W1[c] * (sigma[c] * W1_SCALE)).astype(E4M3)
        b1p = (
            b1[c].astype(np.float64) + mu[c].astype(np.float64) @ W1[c].astype(np.float64)
        ).astype(np.float32)
        in_maps.append(
            {
                "epsT": epsT,
                "w1": _pack_blocks(w1p, LAT_DIM // 128),
                "b1": np.ascontiguousarray(b1p.reshape(NB_NEUR // 128, 128).T),
                "w2": _pack_blocks(W2[c].astype(NPBF16), NB_NEUR // 128),
                "b2": np.ascontiguousarray(b2[c].reshape(OUT_DIM // 128, 128).T),
            }
        )

    res = run_bass_kernel_spmd(
        nc,
        in_maps,
        core_ids=list(range(N_CORES)),
        trace=_trace,
        trace_cores=list(range(N_CORES)) if _trace else None,
    )

    out = np.zeros((n, OUT_DIM), dtype=np.float32)
    for c in range(NB_COMP):
        sel = sels[c]
        if len(sel):
            out[sel] = res.results[c]["yT"][:, : len(sel)].T
    if _trace:
        return out, res
    return out
